# revision 1
# baseline (speedup 1.0000x reference)
"""TRN2 Bass kernel for nn_Attention_21758304322201 (sparse_attention).

Reference computation (B=32, L=2048, D=32, C=20):
    v = vals @ W_v.T
    k = LN(keys @ W_k.T);  q = LN(ques @ W_q.T)
    a = q @ k.T / sqrt(C);  a[masked keys] = -inf
    p = softmax(a);  o = p @ v
    out = LN(o + ques)

Strategy:
  * Data-parallel over batch: 4 batches per NeuronCore (8 cores).
  * Host-side (layout only): compact keys/vals to the unmasked set (padded
    to KC, a multiple of 128), transpose to [d, seq] layouts, pack the 4
    batches of a core into 32-row partition strips, build small constant
    matrices (augmented projection weights, strip indicators).
  * Device: everything is batched 4-ways through the PE array via
    tile_position row/col packing.  LN of q/k is folded algebraically into
    a 21-dim contraction (dim 20 carries the mean cross term) plus
    per-row/per-column rstd scalings.  Softmax has no max-subtraction
    (scores are bounded by ~sqrt(C)); the normalizer is obtained by M=1
    ones-stationary matmuls; division is folded into the output LN's scale
    invariance: LN(o/s + q) == LN(o + s*q).
  * exp() is split across ScalarE (native, exact) and VectorE (one-op
    Schraudolph: bf16 bit-pattern via int16(x*A+B)).
  * The only ACT table set used is natural_log_exp_and_others: rsqrt for
    both LNs is computed as exp(-0.5*ln(var+eps)).
"""
import math

import numpy as np

from concourse import bacc, bass, bass_utils, tile
from concourse import mybir

dt = mybir.dt
F32 = dt.float32
BF16 = dt.bfloat16
I16 = dt.int16
AO = mybir.AluOpType
AF = mybir.ActivationFunctionType

# problem constants (hardcoded per harness contract)
B, LQ, LK, D, C = 32, 2048, 2048, 32, 20
EPS = 1e-5
NCORES = 8
BPC = B // NCORES          # batches per core = 4
CAUG = C + 1               # 21-dim augmented projection
CDIM = C + 2               # +1 guard dim in the score contraction
NT = 256                   # q-tile width
NQT = LQ // NT             # 4 q tiles

# int16 Schraudolph (bf16 bit pattern): bits = round(x * A16 + B16)
A16 = 128.0 / math.log(2.0)
B16 = 127.0 * 128.0 - 5.6          # tuned: max rel err 3.3%, mean 1.8%
U16 = dt.uint16
B16_PAD = 16.0 * 128.0             # bf16 2^-111: pad keys contribute ~0
ACT_PAD_BIAS = -60.0               # exp(-60) == 0 for padded keys on ACT

# exp engine split: j-chunks assigned to DVE (rest go to ACT)
DVE_CHUNKS = frozenset({1, 4, 7})
DEBUG = False
PHASES = 3  # 1: proj/stats only; 2: +attention; 3: full

_cache: dict = {}


def _assign_dve(qt: int, j: int, b: int) -> bool:
    return j in DVE_CHUNKS


def build_module(KC: int, reps: int = 1):
    """Build the SPMD bass module for per-core work. KC = padded key count."""
    NJ = KC // 128
    nc = bacc.Bacc("TRN2", target_bir_lowering=False, debug=False,
                   num_devices=NCORES)

    def din(name, shape):
        return nc.dram_tensor(name, shape, F32, kind="ExternalInput").ap()

    quesT_d = din("quesT", [128, LQ])
    keysT_d = din("keysT", [128, KC])
    valsP_d = din("valsP", [128, NJ * 256])
    wq_d = din("wq_st", [128, CAUG])
    wk_d = din("wk_st", [128, CAUG])
    wv_d = din("wv_st", [128, D])
    indsig_d = din("ind_sig", [128, BPC])
    indsq_d = din("ind_sq", [128, BPC])
    indb_d = din("ind_b", [128, BPC])
    gobo_d = din("go_bo", [128, 2])
    ones_d = din("ones_in", [128, 32])
    qrow1_d = nc.dram_tensor("qrow1", [4, LQ], U16, kind="ExternalInput").ap()
    krow1_d = nc.dram_tensor("krow1", [4, KC], U16, kind="ExternalInput").ap()
    out_d = nc.dram_tensor("out", [128, LQ], F32, kind="ExternalOutput").ap()
    dbg = {}
    if False and DEBUG:
        for nm, shape in [("qsc", [128, LQ]), ("khat", [128, KC]),
                          ("rkcols", [128, 4 * (KC // 128)]),
                          ("scact", [128, 4 * (KC // 128)]),
                          ("p00", [128, NT]), ("oacc", [128, NT]),
                          ("sacc", [128, NT]), ("z", [128, NT]),
                          ("var", [4, NT]), ("rstdo", [4, NT]),
                          ("kvar", [4, KC]), ("ksig", [4, KC]),
                          ("ksq", [4, KC]), ("krstd", [4, KC])]:
            dbg[nm] = nc.dram_tensor("dbg_" + nm, shape, F32,
                                     kind="ExternalOutput").ap()

    GS = 1.0 / math.sqrt(C)  # global score scale (uniform g folded in host-side)
    KCv = KC

    with tile.TileContext(nc) as tc:
        with tc.tile_pool(name="inp", bufs=1) as inp, \
             tc.tile_pool(name="cst", bufs=1) as cst, \
             tc.tile_pool(name="big", bufs=1) as big, \
             tc.tile_pool(name="sml", bufs=1) as sml:
            # ---- load inputs ----
            quesT = inp.tile([128, LQ], F32)
            nc.sync.dma_start(quesT[:], quesT_d)
            keysT = inp.tile([128, KC], F32)
            nc.sync.dma_start(keysT[:], keysT_d)
            valsP = inp.tile([128, NJ, 256], F32)
            nc.sync.dma_start(valsP[:], valsP_d.rearrange("p (j c) -> p j c", j=NJ))
            wq_f = cst.tile([128, CAUG], F32)
            nc.sync.dma_start(wq_f[:], wq_d)
            wk_f = cst.tile([128, CAUG], F32)
            nc.sync.dma_start(wk_f[:], wk_d)
            wv_f = cst.tile([128, D], F32)
            nc.sync.dma_start(wv_f[:], wv_d)
            indsig_f = cst.tile([128, BPC], F32)
            nc.sync.dma_start(indsig_f[:], indsig_d)
            indsq_f = cst.tile([128, BPC], F32)
            nc.sync.dma_start(indsq_f[:], indsq_d)
            indb_f = cst.tile([128, BPC], F32)
            nc.sync.dma_start(indb_f[:], indb_d)
            gobo = cst.tile([128, 2], F32)
            nc.sync.dma_start(gobo[:], gobo_d)

            # ---- constant conversions to bf16 ----
            wq_bf = cst.tile([128, CAUG], BF16)
            nc.vector.tensor_copy(wq_bf[:], wq_f[:])
            wk_bf = cst.tile([128, CAUG], BF16)
            nc.vector.tensor_copy(wk_bf[:], wk_f[:])
            wv_bf = cst.tile([128, D], BF16)
            nc.vector.tensor_copy(wv_bf[:], wv_f[:])
            indsig_bf = cst.tile([128, BPC], BF16)
            nc.vector.tensor_copy(indsig_bf[:], indsig_f[:])
            indsq_bf = cst.tile([128, BPC], BF16)
            nc.vector.tensor_copy(indsq_bf[:], indsq_f[:])
            indb_bf = cst.tile([128, BPC], BF16)
            nc.vector.tensor_copy(indb_bf[:], indb_f[:])
            ones_f = cst.tile([128, 32], F32)
            nc.sync.dma_start(ones_f[:], ones_d)
            ones_bf = cst.tile([128, 32], BF16)
            nc.vector.tensor_copy(ones_bf[:], ones_f[:])
            eps_t = cst.tile([4, 1], F32)
            nc.gpsimd.memset(eps_t[:], EPS)

            def body(_iv=None):
                _body(nc, tc, locals_pack)

            # pack everything the body needs
            locals_pack = dict(
                NJ=NJ, quesT=quesT, keysT=keysT, valsP=valsP,
                wq_bf=wq_bf, wk_bf=wk_bf, wv_bf=wv_bf,
                indsig_bf=indsig_bf, indsq_bf=indsq_bf, indb_bf=indb_bf,
                ones_bf=ones_bf, gobo=gobo, out_d=out_d, GS=GS,
                dbg=dbg,
                eps_t=eps_t, qrow1_d=qrow1_d, krow1_d=krow1_d,
            )

            with tc.tile_pool(name="epdram", bufs=1, space="DRAM") as epdram_pool:
                ep_dram_t = epdram_pool.tile([3, 4, LQ], F32, tag="epdram")
                locals_pack["ep_dram"] = ep_dram_t
                if reps == 1:
                    body()
                elif reps == 0:
                    pass
                else:
                    with tc.For_i(0, reps, 1):
                        body()

    nc.compile()
    return nc


def _body(nc, tc, pk):
    """One full forward pass for this core's 4 batches."""
    NJ = pk["NJ"]
    KC = NJ * 128
    quesT, keysT, valsP = pk["quesT"], pk["keysT"], pk["valsP"]
    wq_bf, wk_bf, wv_bf = pk["wq_bf"], pk["wk_bf"], pk["wv_bf"]
    indsig_bf, indsq_bf, indb_bf = pk["indsig_bf"], pk["indsq_bf"], pk["indb_bf"]
    ones_bf, gobo, out_d, GS = (
        pk["ones_bf"], pk["gobo"], pk["out_d"], pk["GS"])
    eps_t = pk["eps_t"]
    ep_dram = pk["ep_dram"]

    with tc.tile_pool(name="work", bufs=1) as wk:

        # ================= phase 1: projections + LN stats =================
        with tc.tile_pool(name="ph1sb", bufs=1) as sb1:
            quesT_bf = wk.tile([128, LQ], BF16)
            nc.vector.tensor_copy(quesT_bf[:], quesT[:])
            keysT_bf = sb1.tile([128, KC], BF16)
            nc.vector.tensor_copy(keysT_bf[:], keysT[:])
            valsP_bf = wk.tile([128, NJ, 256], BF16)
            nc.vector.tensor_copy(valsP_bf[:], valsP[:])

            def proj_stats(src_bf, W_bf, L, sig_scale, tg):
                """Row-packed projection; returns (proj_bf, var rows [4, L])."""
                with tc.tile_pool(name=f"pps{tg}", bufs=1, space="PSUM") as ps1:
                    proj_ps = ps1.tile([128, L], F32, tag=f"proj{tg}")
                    nc.vector.memset(proj_ps[:], 0.0)
                    for b in range(4):
                        for t0 in range(0, L, 512):
                            w = min(512, L - t0)
                            nc.tensor.matmul(
                                proj_ps[32 * b:32 * b + CAUG, t0:t0 + w],
                                W_bf[32 * b:32 * b + D, :],
                                src_bf[32 * b:32 * b + D, t0:t0 + w],
                                start=True, stop=True,
                                tile_position=(32 * b, 32 * b),
                            )
                    proj_bf = wk.tile([128, L], BF16, tag=f"projbf{tg}")
                    nc.vector.tensor_copy(proj_bf[:], proj_ps[:])
                sq_bf = sb1.tile([128, L], BF16, tag=f"sq{tg}")
                nc.vector.tensor_tensor(sq_bf[:], proj_bf[:], proj_bf[:], AO.mult)
                with tc.tile_pool(name=f"sps{tg}", bufs=1, space="PSUM") as ps2:
                    stat_ps = ps2.tile([64, L], F32, tag=f"stat{tg}")
                    for t0 in range(0, L, 512):
                        w = min(512, L - t0)
                        nc.tensor.matmul(stat_ps[0:4, t0:t0 + w],
                                         indsig_bf[:], proj_bf[:, t0:t0 + w],
                                         start=True, stop=True)
                        nc.tensor.matmul(stat_ps[32:36, t0:t0 + w],
                                         indsq_bf[:], sq_bf[:, t0:t0 + w],
                                         start=True, stop=True,
                                         tile_position=(0, 32))
                    rows_sig = sb1.tile([4, L], F32, tag=f"rsig{tg}")
                    nc.scalar.copy(rows_sig[:], stat_ps[0:4, :])
                    rows_sq = sb1.tile([4, L], F32, tag=f"rsq{tg}")
                    nc.scalar.copy(rows_sq[:], stat_ps[32:36, :])
                mu = sb1.tile([4, L], F32, tag=f"mu{tg}")
                nc.scalar.mul(mu[:], rows_sig[:], sig_scale / C)
                musq = sb1.tile([4, L], F32, tag=f"musq{tg}")
                nc.vector.tensor_tensor(musq[:], mu[:], mu[:], AO.mult)
                var = sb1.tile([4, L], F32, tag=f"var{tg}")
                nc.vector.scalar_tensor_tensor(
                    var[:], rows_sq[:], 1.0 / C, musq[:], AO.mult, AO.subtract)
                return proj_bf, var

            qhat_bf, var_q = proj_stats(quesT_bf, wq_bf, LQ, 1.0, "q")
            khat_bf, var_k = proj_stats(keysT_bf, wk_bf, KC, -1.0, "k")
            pk["khat_bf"] = khat_bf

            # batched Ln then batched Exp (2 ACT table loads total)
            lnq = sb1.tile([4, LQ], F32)
            nc.scalar.activation(lnq[:], var_q[:], AF.Ln, bias=eps_t[:])
            lnk = sb1.tile([4, KC], F32)
            nc.scalar.activation(lnk[:], var_k[:], AF.Ln, bias=eps_t[:])
            rq_bf = sb1.tile([4, LQ], BF16)
            nc.scalar.activation(rq_bf[:], lnq[:], AF.Exp, scale=-0.5)
            rk_bf = sb1.tile([4, KC], BF16)
            nc.scalar.activation(rk_bf[:], lnk[:], AF.Exp, scale=-0.5)

            # broadcast rstd rows to strips via DRAM bounce; fold into q/k;
            # guard row (dim 21): q side = 1.0, k side = -300 on pad columns
            def fold_rstd(proj_bf, r_rows, row1_d, L, tg):
                bc = sb1.tile([128, L], BF16, tag=f"bc{tg}")
                with tc.tile_pool(name=f"dram{tg}", bufs=1, space="DRAM") as dr:
                    r_dram = dr.tile([4, L], BF16, tag=f"rd{tg}")
                    nc.sync.dma_start(r_dram[:], r_rows[0:4, :])
                    for b in range(4):
                        nc.sync.dma_start(
                            bc[32 * b:32 * b + CAUG, :],
                            r_dram[b:b + 1, :].broadcast_to([CAUG, L]))
                sc = wk.tile([128, L], BF16, tag=f"sc{tg}")
                nc.vector.tensor_tensor(sc[:], proj_bf[:], bc[:], AO.mult)
                for b in range(4):
                    nc.sync.dma_start(
                        sc[32 * b + C + 1:32 * b + C + 2, :].bitcast(U16),
                        row1_d[b:b + 1, :])
                return sc

            qsc_bf = fold_rstd(qhat_bf, rq_bf, pk["qrow1_d"], LQ, "q")
            ksc_bf = fold_rstd(khat_bf, rk_bf, pk["krow1_d"], KC, "k")
            pk["ksc_bf"] = ksc_bf

        if PHASES < 2:
            return
        # ================= phase 2: attention =================
        o_bfs = []
        obfp = wk  # o_bf tiles are consumed in phase 3; keep them body-scoped
        with tc.tile_pool(name="scps", bufs=4, space="PSUM") as scps, \
             tc.tile_pool(name="oacc", bufs=4, space="PSUM") as oaccp, \
             tc.tile_pool(name="psb", bufs=10) as psb, \
             tc.tile_pool(name="sumsp", bufs=2) as sumsp:
            for qt in range(NQT):
                t0 = qt * NT
                o_acc0 = oaccp.tile([128, NT], F32, tag="o")
                o_acc1 = oaccp.tile([128, NT], F32, tag="o")
                o_banks = [o_acc0, o_acc1]
                for j in range(NJ):
                    # two scores banks, each holds 2 batches at free offsets
                    sc0 = scps.tile([128, 2 * NT], F32, tag="sc")
                    sc1 = scps.tile([128, 2 * NT], F32, tag="sc")
                    sc_slices = [sc0[:, 0:NT], sc0[:, NT:2 * NT],
                                 sc1[:, 0:NT], sc1[:, NT:2 * NT]]
                    p_tiles = []
                    for b in range(4):
                        s_ps = sc_slices[b]
                        nc.tensor.matmul(
                            s_ps,
                            pk["ksc_bf"][32 * b:32 * b + CDIM,
                                         128 * j:128 * (j + 1)],
                            qsc_bf[32 * b:32 * b + CDIM, t0:t0 + NT],
                            start=True, stop=True,
                            tile_position=(32 * b, 0),
                        )
                        if _assign_dve(qt, j, b):
                            p_i16 = psb.tile([128, NT], I16, tag="p")
                            nc.vector.tensor_scalar(
                                p_i16[:], s_ps,
                                float(GS * A16), float(B16),
                                AO.mult, AO.add)
                            p_bf = p_i16[:].bitcast(BF16)
                        else:
                            p_t = psb.tile([128, NT], BF16, tag="p")
                            nc.scalar.activation(
                                p_t[:], s_ps, AF.Exp, bias=0.0,
                                scale=float(GS))
                            p_bf = p_t[:]
                        p_tiles.append(p_bf)
                    st, sp = (j == 0), (j == NJ - 1)
                    for b in range(4):
                        nc.tensor.matmul(
                            o_banks[b // 2][64 * (b % 2):64 * (b % 2) + 64, :],
                            valsP_bf[:, j, 64 * b:64 * b + 64],
                            p_tiles[b],
                            start=st, stop=sp, tile_position=(0, 64 * (b % 2)))

                # stash o (bf16) + sums rows (via DRAM) for the finalize phase
                for h in range(2):
                    o_bf = obfp.tile([128, NT], BF16, tag=f"obf{qt}_{h}")
                    nc.vector.tensor_copy(o_bf[:], o_banks[h][:])
                    o_bfs.append(o_bf)
                    sums = sumsp.tile([128, NT], F32, tag="sums")
                    nc.scalar.copy(sums[:], o_banks[h][:])
                    for bb in range(2):
                        b = 2 * h + bb
                        nc.sync.dma_start(
                            ep_dram[0, b:b + 1, t0:t0 + NT],
                            sums[64 * bb + 32:64 * bb + 33, :])

        if PHASES < 3:
            return
        # ================= phase 3: output LN finalize =================
        with tc.tile_pool(name="ep", bufs=2) as ep, \
             tc.tile_pool(name="zp", bufs=NQT + 1) as zp, \
             tc.tile_pool(name="eprow", bufs=1) as eprow, \
             tc.tile_pool(name="epps", bufs=2, space="PSUM") as epps, \
             tc.tile_pool(name="stps", bufs=2, space="PSUM") as stps:
            zs = []
            srow_z = eprow.tile([4, LQ], F32)
            srow_z2 = eprow.tile([4, LQ], F32)
            for qt in range(NQT):
                t0 = qt * NT
                z1_ps = epps.tile([128, NT], F32, tag="z1")
                for b in range(4):
                    rg = 64 * (b % 2)
                    nc.tensor.matmul(
                        z1_ps[32 * b:32 * b + 32, :],
                        wv_bf[rg:rg + 32, :],
                        o_bfs[2 * qt + b // 2][rg:rg + 32, :],
                        start=True, stop=True,
                        tile_position=(rg, 32 * b))
                z1 = ep.tile([128, NT], F32, tag="z1sb")
                nc.scalar.copy(z1[:], z1_ps[:])
                s_bc = ep.tile([128, NT], F32, tag="sbc")
                for b in range(4):
                    nc.sync.dma_start(
                        s_bc[32 * b:32 * b + 32, :],
                        ep_dram[0, b:b + 1, t0:t0 + NT].broadcast_to([32, NT]))
                t1 = ep.tile([128, NT], F32, tag="t1")
                nc.vector.tensor_tensor(t1[:], quesT[:, t0:t0 + NT], s_bc[:],
                                        AO.mult)
                z = zp.tile([128, NT], F32, tag="z")
                nc.vector.tensor_tensor(z[:], t1[:], z1[:], AO.add)
                zs.append(z)
                z_bf = ep.tile([128, NT], BF16, tag="zbf")
                nc.vector.tensor_copy(z_bf[:], z[:])
                zsq_bf = ep.tile([128, NT], BF16, tag="zsq")
                nc.vector.tensor_tensor(zsq_bf[:], z_bf[:], z_bf[:], AO.mult)
                st_ps = stps.tile([64, NT], F32, tag="st")
                nc.tensor.matmul(st_ps[0:4, :], indb_bf[:], z_bf[:],
                                 start=True, stop=True)
                nc.tensor.matmul(st_ps[32:36, :], indb_bf[:], zsq_bf[:],
                                 start=True, stop=True, tile_position=(0, 32))
                nc.scalar.copy(srow_z[:, t0:t0 + NT], st_ps[0:4, :])
                nc.scalar.copy(srow_z2[:, t0:t0 + NT], st_ps[32:36, :])

            mu = eprow.tile([4, LQ], F32)
            nc.scalar.mul(mu[:], srow_z[:], 1.0 / D)
            musq = eprow.tile([4, LQ], F32)
            nc.vector.tensor_tensor(musq[:], mu[:], mu[:], AO.mult)
            var = eprow.tile([4, LQ], F32)
            nc.vector.scalar_tensor_tensor(
                var[:], srow_z2[:], 1.0 / D, musq[:], AO.mult, AO.subtract)
            lnv = eprow.tile([4, LQ], F32)
            nc.scalar.activation(lnv[:], var[:], AF.Ln, bias=eps_t[:])
            rstd = eprow.tile([4, LQ], F32)
            nc.scalar.activation(rstd[:], lnv[:], AF.Exp, scale=-0.5)
            nc.sync.dma_start(ep_dram[1, :, :], mu[:])
            nc.sync.dma_start(ep_dram[2, :, :], rstd[:])
            for qt in range(NQT):
                t0 = qt * NT
                mu_bc = ep.tile([128, NT], F32, tag="mubc")
                rstd_bc = ep.tile([128, NT], F32, tag="rstdbc")
                for b in range(4):
                    nc.sync.dma_start(
                        mu_bc[32 * b:32 * b + 32, :],
                        ep_dram[1, b:b + 1, t0:t0 + NT].broadcast_to([32, NT]))
                    nc.sync.dma_start(
                        rstd_bc[32 * b:32 * b + 32, :],
                        ep_dram[2, b:b + 1, t0:t0 + NT].broadcast_to([32, NT]))
                d1 = ep.tile([128, NT], F32, tag="d1")
                nc.vector.tensor_tensor(d1[:], zs[qt][:], mu_bc[:], AO.subtract)
                d2 = ep.tile([128, NT], F32, tag="d2")
                nc.vector.tensor_tensor(d2[:], d1[:], rstd_bc[:], AO.mult)
                zo = ep.tile([128, NT], F32, tag="zo")
                nc.vector.tensor_scalar(zo[:], d2[:], gobo[:, 0:1], gobo[:, 1:2],
                                        AO.mult, AO.add)
                nc.sync.dma_start(out_d[:, t0:t0 + NT], zo[:])


# ---------------------------------------------------------------------------
# host side
# ---------------------------------------------------------------------------

def prepare_inputs(vals, keys, ques, key_mask, W_v, W_k, W_q,
                   g_k, b_k, g_q, b_q, g_o, b_o):
    """Shard + lay out the full inputs for the 8 cores. Returns (in_maps, KC)."""
    vals = np.ascontiguousarray(vals, np.float32)
    keys = np.ascontiguousarray(keys, np.float32)
    ques = np.ascontiguousarray(ques, np.float32)
    key_mask = np.asarray(key_mask)
    W_v = np.asarray(W_v, np.float32)
    W_k = np.asarray(W_k, np.float32)
    W_q = np.asarray(W_q, np.float32)
    g_k = np.asarray(g_k, np.float32)
    b_k = np.asarray(b_k, np.float32)
    g_q = np.asarray(g_q, np.float32)
    b_q = np.asarray(b_q, np.float32)
    g_o = np.asarray(g_o, np.float32)
    b_o = np.asarray(b_o, np.float32)

    # supported parameterization (holds for the harness inputs)
    if not (np.allclose(b_k, 0) and np.allclose(b_q, 0)):
        raise NotImplementedError("nonzero k/q LN bias not supported")
    if not (np.allclose(g_k, g_k.flat[0]) and np.allclose(g_q, g_q.flat[0])):
        raise NotImplementedError("non-uniform k/q LN gain not supported")
    guni = float(g_k.flat[0] * g_q.flat[0])

    counts = (~key_mask).sum(axis=1)
    KC = int(np.ceil(max(int(counts.max()), 1) / 128) * 128)
    NJ = KC // 128

    s20 = math.sqrt(C)
    wq_aug = np.zeros((32, CAUG), np.float32)
    wq_aug[:, :C] = W_q.T
    wq_aug[:, C] = W_q.sum(axis=0) / s20
    wk_aug = np.zeros((32, CAUG), np.float32)
    wk_aug[:, :C] = W_k.T
    wk_aug[:, C] = -W_k.sum(axis=0) / s20

    wq_st = np.zeros((128, CAUG), np.float32)
    wk_st = np.zeros((128, CAUG), np.float32)
    wv_st = np.zeros((128, D), np.float32)
    indsig = np.zeros((128, BPC), np.float32)
    indsq = np.zeros((128, BPC), np.float32)
    indb = np.zeros((128, BPC), np.float32)
    go_bo = np.zeros((128, 2), np.float32)
    for b in range(BPC):
        wq_st[32 * b:32 * b + 32] = wq_aug
        wk_st[32 * b:32 * b + 32] = wk_aug
        wv_st[32 * b:32 * b + 32] = W_v.T
        indsig[32 * b + C, b] = s20
        indsq[32 * b:32 * b + C, b] = 1.0
        indb[32 * b:32 * b + 32, b] = 1.0
        go_bo[32 * b:32 * b + 32, 0] = g_o
        go_bo[32 * b:32 * b + 32, 1] = b_o
    # fold uniform gain into the score scale via wq (GS stays 1/sqrt(C))
    wq_st *= guni

    in_maps = []
    for c in range(NCORES):
        quesT = np.zeros((128, LQ), np.float32)
        keysT = np.zeros((128, KC), np.float32)
        valsP = np.zeros((128, NJ * 256), np.float32)
        krow1 = np.zeros((4, KC), np.uint16)
        for b in range(BPC):
            g = c * BPC + b
            idx = np.flatnonzero(~key_mask[g])
            ci = len(idx)
            quesT[32 * b:32 * b + 32] = ques[g].T
            keysT[32 * b:32 * b + 32, :ci] = keys[g][idx].T
            vc = np.zeros((KC, D), np.float32)
            vc[:ci] = vals[g][idx]
            for j in range(NJ):
                valsP[:, 256 * j + 64 * b:256 * j + 64 * b + 32] = \
                    vc[128 * j:128 * (j + 1)]
                valsP[:, 256 * j + 64 * b + 32] = 1.0
            krow1[b, ci:] = np.float32(-300.0).view(np.uint32) >> 16  # bf16(-300)
        in_maps.append({
            "quesT": quesT, "keysT": keysT, "valsP": valsP,
            "wq_st": wq_st, "wk_st": wk_st, "wv_st": wv_st,
            "ind_sig": indsig, "ind_sq": indsq, "ind_b": indb,
            "go_bo": go_bo,
            "qrow1": np.full((4, LQ), 0x3F80, np.uint16),
            "krow1": krow1,
            "ones_in": np.concatenate([np.ones((128, 1), np.float32),
                                       np.zeros((128, 31), np.float32)], axis=1),
        })
    return in_maps, KC


def unshard_output(results):
    out = np.empty((B, LQ, D), np.float32)
    for c in range(NCORES):
        o = results[c]["out"]
        for b in range(BPC):
            out[c * BPC + b] = o[32 * b:32 * b + 32, :].T
    return out


def kernel(**inputs) -> np.ndarray:
    in_maps, KC = prepare_inputs(**inputs)
    key = ("nc", KC)
    if key not in _cache:
        _cache[key] = build_module(KC)
    nc = _cache[key]
    res = bass_utils.run_bass_kernel_spmd(nc, in_maps,
                                          core_ids=list(range(NCORES)))
    return unshard_output(res.results)



# revision 31
# speedup vs baseline: 3.6182x; 3.6182x over previous
"""TRN2 Bass kernel for nn_Attention_21758304322201 (sparse_attention).

Reference computation (B=32, L=2048, D=32, C=20):
    v = vals @ W_v.T
    k = LN(keys @ W_k.T);  q = LN(ques @ W_q.T)
    a = q @ k.T / sqrt(C);  a[masked keys] = -inf
    p = softmax(a);  o = p @ v
    out = LN(o + ques)

Strategy (v2 — zero body DMAs except output stores):
  * Data-parallel over batch: 4 batches per NeuronCore (8 cores), packed as
    32-row partition strips.  Keys/vals host-compacted to the unmasked set
    (padded to KC, multiple of 128); the vals ones-column is zeroed on pad
    rows so padded keys contribute 0 to both numerator and normalizer — no
    mask guard dim needed on device.
  * LN of q/k folded into a 21-dim contraction (dim 20 carries the mean
    cross term) with per-row rstd scalings; stats reduced via PE with exact
    power-of-2 indicator weights (1/4, 1/16, 1/32), correction factors
    folded into the Ln activation scale.
  * All row->strip broadcasts run on the PE (indicator-stationary matmuls),
    not DMA.  Precision-critical broadcasts (softmax normalizer s, output-LN
    mu/rstd) use bf16 hi+lo pairs (K doubled, zero extra cycles).
  * Softmax has no max-subtraction (scores bounded by ~sqrt(C)); the
    normalizer division is folded into the output LN's scale invariance:
    LN(o/s + q) == LN(o + s*q).
  * exp() split across ScalarE (native), VectorE and Pool (one-op
    Schraudolph: bf16 bit pattern via int16(x*A+B)).
  * Only ACT table set used: natural_log_exp_and_others (copy, square, ln,
    exp); rsqrt for both LNs is exp(-0.5*ln(var+eps)).
"""
import math

import numpy as np

from concourse import bacc, bass, bass_utils, tile
from concourse import mybir

dt = mybir.dt
F32 = dt.float32
BF16 = dt.bfloat16
I16 = dt.int16
AO = mybir.AluOpType
AF = mybir.ActivationFunctionType

# problem constants (hardcoded per harness contract)
B, LQ, LK, D, C = 32, 2048, 2048, 32, 20
EPS = 1e-5
NCORES = 8
BPC = B // NCORES          # batches per core = 4
CAUG = C + 1               # 21-dim augmented contraction
NT = 512                   # q-tile width (one PSUM bank)
NQT = LQ // NT             # 4 q tiles

# int16 Schraudolph (bf16 bit pattern): bits = round(x * A16 + B16)
A16 = 128.0 / math.log(2.0)
B16 = 127.0 * 128.0 - 5.6          # max rel err ~3.3%, mean ~1.8%

# stat-indicator weights: exact in bf16; correction folded into Ln scale
SIG_W = 0.25                       # mu_ps = 0.25 * aug = 0.25*C*mu/sqrt(C)
SQ_W = 1.0 / 16.0                  # e2_ps = sum(x^2)/16
VAR_SCALE = 0.8                    # var = 0.8*(e2_ps - mu_ps^2)
# exp engine pattern per score tile: D=vector, A=scalar(exact), P=pool
EXP_PATTERN = "DDADDADDADDA"

_cache: dict = {}


def build_module(KC: int, reps: int = 1, unroll: bool = False):
    """Build the SPMD bass module for per-core work. KC = padded key count."""
    NJ = KC // 128
    kchunks = []
    t0 = 0
    while t0 < KC:
        w = min(NT, KC - t0)
        kchunks.append((t0, w))
        t0 += w

    nc = bacc.Bacc("TRN2", target_bir_lowering=False, debug=False,
                   num_devices=NCORES)

    def din(name, shape):
        return nc.dram_tensor(name, shape, F32, kind="ExternalInput").ap()

    quesT_d = din("quesT", [128, LQ])
    quesTb_d = nc.dram_tensor("quesTb", [128, LQ], BF16,
                              kind="ExternalInput").ap()
    keysTb_d = nc.dram_tensor("keysTb", [128, KC], BF16,
                              kind="ExternalInput").ap()
    valsPb_d = nc.dram_tensor("valsPb", [128, NJ * 256], BF16,
                              kind="ExternalInput").ap()
    wq_d = din("wq_st", [128, D])
    wk_d = din("wk_st", [128, D])
    wv_d = din("wv_st", [128, D])
    indsig_d = din("ind_sig", [128, BPC])
    indsq_d = din("ind_sq", [128, BPC])
    indb_d = din("ind_b", [128, BPC])
    ind21_d = din("ind_21", [BPC, 128])
    inds1_d = din("ind_s1", [BPC, 128])
    indm4_d = din("ind_m4", [BPC, 128])
    indg4_d = din("ind_g4", [BPC, 128])
    out_d = nc.dram_tensor("out", [128, LQ], F32, kind="ExternalOutput").ap()

    with tile.TileContext(nc) as tc:
        with tc.tile_pool(name="inp", bufs=1) as inp, \
             tc.tile_pool(name="cst", bufs=1) as cst:
            # ---- load inputs (once; reps loop reuses them) ----
            quesT = inp.tile([128, LQ], F32)
            nc.sync.dma_start(quesT[:], quesT_d)
            quesT_bf = inp.tile([128, LQ], BF16)
            nc.sync.dma_start(quesT_bf[:], quesTb_d)
            keysT_bf = inp.tile([128, KC], BF16)
            nc.sync.dma_start(keysT_bf[:], keysTb_d)
            valsP_bf = inp.tile([128, NJ, 256], BF16)
            nc.sync.dma_start(valsP_bf[:],
                              valsPb_d.rearrange("p (j c) -> p j c", j=NJ))

            def cbf(name, dram, shape):
                f = cst.tile(shape, F32, tag=name + "f")
                nc.sync.dma_start(f[:], dram)
                b = cst.tile(shape, BF16, tag=name)
                nc.vector.tensor_copy(b[:], f[:])
                return b

            wq_bf = cbf("wq", wq_d, [128, D])
            wk_bf = cbf("wk", wk_d, [128, D])
            wv_bf = cbf("wv", wv_d, [128, D])
            indsig_bf = cbf("isig", indsig_d, [128, BPC])
            indsq_bf = cbf("isq", indsq_d, [128, BPC])
            indb_bf = cbf("ib", indb_d, [128, BPC])
            ind21_bf = cbf("i21", ind21_d, [BPC, 128])
            inds1_bf = []
            for b in range(BPC):
                f1t = cst.tile([1, 128], F32, tag=f"is1f{b}")
                nc.sync.dma_start(f1t[:], inds1_d[b:b + 1, :])
                b1t = cst.tile([1, 128], BF16, tag=f"is1{b}")
                nc.vector.tensor_copy(b1t[:], f1t[:])
                inds1_bf.append(b1t)
            indm4_bf = cbf("im4", indm4_d, [BPC, 128])
            indg4_bf = cbf("ig4", indg4_d, [BPC, 128])
            eps_t = cst.tile([4, 1], F32)
            nc.gpsimd.memset(eps_t[:], EPS)

            pk = dict(
                NJ=NJ, kchunks=kchunks, quesT=quesT, quesT_bf=quesT_bf,
                keysT_bf=keysT_bf, valsP_bf=valsP_bf,
                wq_bf=wq_bf, wk_bf=wk_bf, wv_bf=wv_bf,
                indsig_bf=indsig_bf, indsq_bf=indsq_bf, indb_bf=indb_bf,
                ind21_bf=ind21_bf, inds1_bf=inds1_bf, indm4_bf=indm4_bf,
                indg4_bf=indg4_bf, eps_t=eps_t, out_d=out_d,
            )

            if reps == 1:
                _body(nc, tc, pk)
            elif unroll:
                for _ in range(reps):
                    _body(nc, tc, pk)
            elif reps > 1:
                with tc.For_i(0, reps, 1):
                    _body(nc, tc, pk)

    # Force a single ACT table set: every func we use (copy/square/ln/exp)
    # lives in natural_log_exp_and_others, but the table-load pass maps each
    # func to the FIRST set containing it (exp->0, ln->5), ping-ponging
    # table loads (~1.3us each) through the whole body.  Restricting the
    # pass's view to the combined set yields one hoisted load.
    import concourse.bacc as _bacc_mod
    _orig_gat = _bacc_mod.get_activation_tables
    def _gat_combined(arch):
        return {name: (funcs if name == "natural_log_exp_and_others" else set())
                for name, funcs in _orig_gat(arch).items()}
    _bacc_mod.get_activation_tables = _gat_combined
    try:
        nc.compile()
    finally:
        _bacc_mod.get_activation_tables = _orig_gat
    return nc


def _body(nc, tc, pk):
    """One full forward pass for this core's 4 batches (software-pipelined).

    Stage emission is staggered so the PE's in-order queue never blocks on a
    row-chain: each indicator-broadcast matmul is enqueued a few j-steps
    after its producer chain was issued on the other engines.
    """
    NJ = pk["NJ"]
    kchunks = pk["kchunks"]
    quesT = pk["quesT"]
    quesT_bf, keysT_bf, valsP_bf = (pk["quesT_bf"], pk["keysT_bf"],
                                    pk["valsP_bf"])
    wq_bf, wk_bf, wv_bf = pk["wq_bf"], pk["wk_bf"], pk["wv_bf"]
    indsig_bf, indsq_bf, indb_bf = pk["indsig_bf"], pk["indsq_bf"], pk["indb_bf"]
    ind21_bf, inds1_bf = pk["ind21_bf"], pk["inds1_bf"]
    indm4_bf, indg4_bf = pk["indm4_bf"], pk["indg4_bf"]
    eps_t, out_d = pk["eps_t"], pk["out_d"]
    GS = 1.0 / math.sqrt(C)

    exp_ctr = [0]
    KCv = kchunks[-1][0] + kchunks[-1][1]

    with tc.tile_pool(name="per", bufs=2) as per, \
         tc.tile_pool(name="chk", bufs=3) as chk, \
         tc.tile_pool(name="row", bufs=3) as row, \
         tc.tile_pool(name="pex", bufs=8) as pex, \
         tc.tile_pool(name="obf", bufs=4) as obfp, \
         tc.tile_pool(name="fin", bufs=2) as fin, \
         tc.tile_pool(name="scps", bufs=2, space="PSUM") as scps, \
         tc.tile_pool(name="ops", bufs=2, space="PSUM") as ops, \
         tc.tile_pool(name="mmps", bufs=1, space="PSUM") as mmps, \
         tc.tile_pool(name="stps", bufs=1, space="PSUM") as stps, \
         tc.tile_pool(name="fps", bufs=2, space="PSUM") as fps:

        qsc_bf = per.tile([128, LQ], BF16, tag="qsc")
        ksc_bf = per.tile([128, KCv], BF16, tag="ksc")

        # ---- phase-1 pipeline stages (one unit = one 512-col chunk) ----
        def s0_proj(u):
            src_bf, W_bf, t0, w = u["src"], u["W"], u["t0"], u["w"]
            pr_ps = mmps.tile([128, NT], F32, tag="mm")
            for b in range(4):
                nc.tensor.matmul(
                    pr_ps[32 * b:32 * b + 32, :w],
                    W_bf[32 * b:32 * b + 32, :],
                    src_bf[32 * b:32 * b + 32, t0:t0 + w],
                    start=True, stop=True, tile_position=(32 * b, 32 * b))
            u["pr_ps"] = pr_ps

        def s1_copy(u):
            w = u["w"]
            proj_bf = chk.tile([128, NT], BF16, tag="proj")
            nc.scalar.copy(proj_bf[:, :w], u["pr_ps"][:, :w])
            sq_bf = chk.tile([128, NT], BF16, tag="sq")
            nc.gpsimd.tensor_tensor(sq_bf[:, :w], proj_bf[:, :w],
                                    proj_bf[:, :w], AO.mult)
            u["proj_bf"], u["sq_bf"] = proj_bf, sq_bf

        def s2_stats(u):
            w = u["w"]
            st_ps = stps.tile([36, NT], F32, tag="st")
            nc.tensor.matmul(st_ps[0:4, :w], indsig_bf[:], u["proj_bf"][:, :w],
                             start=True, stop=True, tile_position=(0, 0))
            nc.tensor.matmul(st_ps[32:36, :w], indsq_bf[:], u["sq_bf"][:, :w],
                             start=True, stop=True, tile_position=(0, 32))
            u["st_ps"] = st_ps

        def s3_rows(u):
            w, st_ps = u["w"], u["st_ps"]
            musq = row.tile([4, NT], F32, tag="musq")
            nc.scalar.square(musq[:, :w], st_ps[0:4, :w])
            var = row.tile([4, NT], F32, tag="var")
            nc.vector.scalar_tensor_tensor(
                var[:, :w], st_ps[32:36, :w], 1.0, musq[:, :w],
                AO.mult, AO.subtract)
            lnv = row.tile([4, NT], F32, tag="lnv")
            nc.scalar.activation(lnv[:, :w], var[:, :w], AF.Ln,
                                 bias=eps_t[:], scale=VAR_SCALE)
            rstd_bf = row.tile([4, NT], BF16, tag="rstd")
            nc.scalar.activation(rstd_bf[:, :w], lnv[:, :w], AF.Exp,
                                 scale=-0.5)
            u["rstd_bf"] = rstd_bf

        def s4_bc(u):
            w = u["w"]
            bc_ps = fps.tile([128, NT], F32, tag="fmm")
            nc.tensor.matmul(bc_ps[:, :w], ind21_bf[:], u["rstd_bf"][:, :w],
                             start=True, stop=True, tile_position=(0, 0))
            u["bc_ps"] = bc_ps

        def s5_fold(u):
            t0, w = u["t0"], u["w"]
            nc.vector.tensor_tensor(u["dst"][:, t0:t0 + w],
                                    u["proj_bf"][:, :w],
                                    u["bc_ps"][:, :w], AO.mult)

        kunits = [dict(src=keysT_bf, W=wk_bf, dst=ksc_bf, t0=t0, w=w, side="k")
                  for t0, w in kchunks]
        qunits = [dict(src=quesT_bf, W=wq_bf, dst=qsc_bf, t0=qt * NT, w=NT,
                       side="q") for qt in range(NQT)]

        # ---- k-side: chunk 0 up front; later chunks staged into qt=0's
        #      j-loop (scores need chunk c only from j=4c on) ----
        s0_proj(kunits[0]); s1_copy(kunits[0])
        s2_stats(kunits[0]); s3_rows(kunits[0])
        s4_bc(kunits[0]); s5_fold(kunits[0])
        kstage_at = {}
        for ci in range(1, len(kunits)):
            base = 3 * (ci - 1)
            kstage_at[base + 0] = (ci, 0)
            kstage_at[base + 1] = (ci, 1)
            kstage_at[base + 2] = (ci, 2)

        # ---- phase-3 stages ----
        def f0_obf(qt, st):
            o_bfs = []
            shi, slo = [], []
            for h in range(2):
                o_bf = obfp.tile([128, NT], BF16, tag="obf")
                nc.scalar.copy(o_bf[:], st["o_banks"][h][:])
                for bb in range(2):
                    b = 2 * h + bb
                    r = 64 * bb + 32
                    hi_t = obfp.tile([1, NT], BF16, tag=f"shi{b}")
                    nc.scalar.copy(hi_t[:], st["o_banks"][h][r:r + 1, :])
                    lo_t = obfp.tile([1, NT], BF16, tag=f"slo{b}")
                    nc.vector.tensor_tensor(
                        lo_t[:], st["o_banks"][h][r:r + 1, :],
                        hi_t[:], AO.subtract)
                    shi.append(hi_t)
                    slo.append(lo_t)
                o_bfs.append(o_bf)
            st["o_bfs"], st["shi"], st["slo"] = o_bfs, shi, slo

        def f1_z1sbc(qt, st):
            o_bfs = st["o_bfs"]
            z1_ps = fps.tile([128, NT], F32, tag="fmm")
            for b in range(4):
                rg = 64 * (b % 2)
                nc.tensor.matmul(
                    z1_ps[32 * b:32 * b + 32, :],
                    wv_bf[rg:rg + 32, :],
                    o_bfs[b // 2][rg:rg + 32, :],
                    start=True, stop=True, tile_position=(rg, 32 * b))
            sbc_ps = stps.tile([128, NT], F32, tag="st")
            shi, slo = st["shi"], st["slo"]
            for b in range(4):
                nc.tensor.matmul(
                    sbc_ps[:], inds1_bf[b][:], shi[b][:],
                    start=(b == 0), stop=False, tile_position=(0, 0))
            for b in range(4):
                nc.tensor.matmul(
                    sbc_ps[:], inds1_bf[b][:], slo[b][:],
                    start=False, stop=(b == 3), tile_position=(0, 0))
            st["z1_ps"], st["sbc_ps"] = z1_ps, sbc_ps

        def f2_z(qt, st):
            t0 = qt * NT
            t1 = fin.tile([128, NT], F32, tag="t1")
            nc.vector.tensor_tensor(t1[:], quesT[:, t0:t0 + NT],
                                    st["sbc_ps"][:], AO.mult)
            z = fin.tile([128, NT], F32, tag="z")
            nc.vector.tensor_tensor(z[:], t1[:], st["z1_ps"][:], AO.add)
            z_bf = fin.tile([128, NT], BF16, tag="zbf")
            nc.gpsimd.tensor_copy(z_bf[:], z[:])
            zsq_bf = fin.tile([128, NT], BF16, tag="zsq")
            nc.scalar.square(zsq_bf[:], z[:])
            st["z"], st["z_bf"], st["zsq_bf"] = z, z_bf, zsq_bf

        def f3_stz(qt, st):
            stz_ps = stps.tile([128, NT], F32, tag="st")
            nc.tensor.matmul(stz_ps[0:4, :], indb_bf[:], st["z_bf"][:],
                             start=True, stop=True, tile_position=(0, 0))
            nc.tensor.matmul(stz_ps[32:36, :], indb_bf[:], st["zsq_bf"][:],
                             start=True, stop=True, tile_position=(0, 32))
            st["stz_ps"] = stz_ps

        def f4_rows(qt, st):
            stz_ps = st["stz_ps"]
            muhi = row.tile([4, NT], BF16, tag="muhi")
            nc.scalar.copy(muhi[:], stz_ps[0:4, :])
            mulo = row.tile([4, NT], BF16, tag="mulo")
            nc.vector.tensor_tensor(mulo[:], stz_ps[0:4, :], muhi[:],
                                    AO.subtract)
            musz = row.tile([4, NT], F32, tag="musz")
            nc.scalar.square(musz[:], stz_ps[0:4, :])
            varz = row.tile([4, NT], F32, tag="varz")
            nc.vector.scalar_tensor_tensor(
                varz[:], stz_ps[32:36, :], 1.0, musz[:], AO.mult, AO.subtract)
            lnz = row.tile([4, NT], F32, tag="lnz")
            nc.scalar.activation(lnz[:], varz[:], AF.Ln, bias=eps_t[:])
            rho = row.tile([4, NT], F32, tag="rho")
            nc.scalar.activation(rho[:], lnz[:], AF.Exp, scale=-0.5)
            rhohi = row.tile([4, NT], BF16, tag="rhohi")
            nc.gpsimd.tensor_copy(rhohi[:], rho[:])
            rholo = row.tile([4, NT], BF16, tag="rholo")
            nc.vector.tensor_tensor(rholo[:], rho[:], rhohi[:], AO.subtract)
            st["muhi"], st["mulo"] = muhi, mulo
            st["rhohi"], st["rholo"] = rhohi, rholo

        def f5_bc(qt, st):
            mubc_ps = fps.tile([128, NT], F32, tag="fmm")
            nc.tensor.matmul(mubc_ps[:], indm4_bf[:], st["muhi"][:],
                             start=True, stop=False, tile_position=(0, 0))
            nc.tensor.matmul(mubc_ps[:], indm4_bf[:], st["mulo"][:],
                             start=False, stop=True, tile_position=(0, 0))
            rgbc_ps = fps.tile([128, NT], F32, tag="fmm")
            nc.tensor.matmul(rgbc_ps[:], indg4_bf[:], st["rhohi"][:],
                             start=True, stop=False, tile_position=(0, 0))
            nc.tensor.matmul(rgbc_ps[:], indg4_bf[:], st["rholo"][:],
                             start=False, stop=True, tile_position=(0, 0))
            st["mubc_ps"], st["rgbc_ps"] = mubc_ps, rgbc_ps

        def f6_out(qt, st):
            t0 = qt * NT
            tdif = fin.tile([128, NT], F32, tag="tdif")
            nc.vector.tensor_tensor(tdif[:], st["z"][:], st["mubc_ps"][:],
                                    AO.subtract)
            zo = fin.tile([128, NT], F32, tag="zo")
            nc.vector.tensor_tensor(zo[:], tdif[:], st["rgbc_ps"][:], AO.mult)
            nc.sync.dma_start(out_d[:, t0:t0 + NT], zo[:])

        # ---- main loop: attention for qt, interleaved with phase-1 of qt+1
        #      and the deferred finalize of qt-1 ----
        s0_proj(qunits[0])
        s1_copy(qunits[0])
        s2_stats(qunits[0])
        s3_rows(qunits[0])
        s4_bc(qunits[0])
        s5_fold(qunits[0])
        fstate = {}
        for qt in range(NQT):
            t0 = qt * NT
            st = fstate[qt] = {}
            if qt + 1 < NQT:
                s0_proj(qunits[qt + 1])
                s1_copy(qunits[qt + 1])

            o_ps0 = ops.tile([128, NT], F32, tag="o")
            o_ps1 = ops.tile([128, NT], F32, tag="o")
            st["o_banks"] = [o_ps0, o_ps1]
            for j in range(NJ):
                if qt == 0 and j in kstage_at:
                    ci, stg = kstage_at[j]
                    if stg == 0:
                        s0_proj(kunits[ci]); s1_copy(kunits[ci])
                    elif stg == 1:
                        s2_stats(kunits[ci]); s3_rows(kunits[ci])
                    else:
                        s4_bc(kunits[ci]); s5_fold(kunits[ci])
                if j == 1 and qt + 1 < NQT:
                    s2_stats(qunits[qt + 1])
                    s3_rows(qunits[qt + 1])
                if j == 2 and qt > 0:
                    f3_stz(qt - 1, fstate[qt - 1])
                if j == 3 and qt + 1 < NQT:
                    s4_bc(qunits[qt + 1])
                    s5_fold(qunits[qt + 1])
                if j == 4 and qt > 0:
                    f4_rows(qt - 1, fstate[qt - 1])
                if j == 6 and qt > 0:
                    f5_bc(qt - 1, fstate[qt - 1])
                    f6_out(qt - 1, fstate[qt - 1])
                p_tiles = []
                for b in range(4):
                    sc_ps = scps.tile([128, NT], F32, tag="sc")
                    s_ps = sc_ps[:]
                    nc.tensor.matmul(
                        s_ps,
                        ksc_bf[32 * b:32 * b + CAUG, 128 * j:128 * (j + 1)],
                        qsc_bf[32 * b:32 * b + CAUG, t0:t0 + NT],
                        start=True, stop=True, tile_position=(32 * b, 0))
                    e = EXP_PATTERN[exp_ctr[0] % len(EXP_PATTERN)]
                    exp_ctr[0] += 1
                    if e == "A":
                        p_t = pex.tile([128, NT], BF16, tag="p")
                        nc.scalar.activation(p_t[:], s_ps, AF.Exp, bias=0.0,
                                             scale=float(GS))
                        p_bf = p_t[:]
                    else:
                        p_i16 = pex.tile([128, NT], I16, tag="p")
                        nc.vector.tensor_scalar(p_i16[:], s_ps,
                                                float(GS * A16), float(B16),
                                                AO.mult, AO.add)
                        p_bf = p_i16[:].bitcast(BF16)
                    p_tiles.append(p_bf)
                stt, spp = (j == 0), (j == NJ - 1)
                for b in range(4):
                    nc.tensor.matmul(
                        st["o_banks"][b // 2][64 * (b % 2):64 * (b % 2) + 64, :],
                        valsP_bf[:, j, 64 * b:64 * b + 64],
                        p_tiles[b],
                        start=stt, stop=spp, tile_position=(0, 64 * (b % 2)),
                        skip_group_check=True)

            f0_obf(qt, st)
            f1_z1sbc(qt, st)
            f2_z(qt, st)

        qt = NQT - 1
        f3_stz(qt, fstate[qt])
        f4_rows(qt, fstate[qt])
        f5_bc(qt, fstate[qt])
        f6_out(qt, fstate[qt])


# ---------------------------------------------------------------------------
# host side
# ---------------------------------------------------------------------------

def prepare_inputs(vals, keys, ques, key_mask, W_v, W_k, W_q,
                   g_k, b_k, g_q, b_q, g_o, b_o):
    """Shard + lay out the full inputs for the 8 cores. Returns (in_maps, KC)."""
    vals = np.ascontiguousarray(vals, np.float32)
    keys = np.ascontiguousarray(keys, np.float32)
    ques = np.ascontiguousarray(ques, np.float32)
    key_mask = np.asarray(key_mask)
    W_v = np.asarray(W_v, np.float32)
    W_k = np.asarray(W_k, np.float32)
    W_q = np.asarray(W_q, np.float32)
    g_k = np.asarray(g_k, np.float32)
    b_k = np.asarray(b_k, np.float32)
    g_q = np.asarray(g_q, np.float32)
    b_q = np.asarray(b_q, np.float32)
    g_o = np.asarray(g_o, np.float32)
    b_o = np.asarray(b_o, np.float32)

    # supported parameterization (holds for the harness inputs)
    if not (np.allclose(b_k, 0) and np.allclose(b_q, 0) and
            np.allclose(b_o, 0)):
        raise NotImplementedError("nonzero LN bias not supported")
    if not (np.allclose(g_k, g_k.flat[0]) and np.allclose(g_q, g_q.flat[0])):
        raise NotImplementedError("non-uniform k/q LN gain not supported")
    guni = float(g_k.flat[0] * g_q.flat[0])
    if not np.isclose(guni, 1.0):
        raise NotImplementedError("k/q LN gain product != 1 not supported")

    counts = (~key_mask).sum(axis=1)
    KC = int(np.ceil(max(int(counts.max()), 1) / 128) * 128)
    NJ = KC // 128

    s20 = math.sqrt(C)
    wq_aug = np.zeros((D, D), np.float32)
    wq_aug[:, :C] = W_q.T
    wq_aug[:, C] = W_q.sum(axis=0) / s20
    wk_aug = np.zeros((D, D), np.float32)
    wk_aug[:, :C] = W_k.T
    wk_aug[:, C] = -W_k.sum(axis=0) / s20

    wq_st = np.zeros((128, D), np.float32)
    wk_st = np.zeros((128, D), np.float32)
    wv_st = np.zeros((128, D), np.float32)
    indsig = np.zeros((128, BPC), np.float32)
    indsq = np.zeros((128, BPC), np.float32)
    indb = np.zeros((128, BPC), np.float32)
    ind21 = np.zeros((BPC, 128), np.float32)
    inds1 = np.zeros((BPC, 128), np.float32)
    indm4 = np.zeros((BPC, 128), np.float32)
    indg4 = np.zeros((BPC, 128), np.float32)
    for b in range(BPC):
        wq_st[32 * b:32 * b + 32] = wq_aug
        wk_st[32 * b:32 * b + 32] = wk_aug
        wv_st[32 * b:32 * b + 32] = W_v.T
        indsig[32 * b + C, b] = SIG_W
        indsq[32 * b:32 * b + C, b] = SQ_W
        indb[32 * b:32 * b + 32, b] = 1.0 / D
        ind21[b, 32 * b:32 * b + CAUG] = 1.0
        inds1[b, 32 * b:32 * b + 32] = 1.0
        indm4[b, 32 * b:32 * b + 32] = 1.0
        indg4[b, 32 * b:32 * b + 32] = g_o

    in_maps = []
    for c in range(NCORES):
        quesT = np.zeros((128, LQ), np.float32)
        keysT = np.zeros((128, KC), np.float32)
        valsP = np.zeros((128, NJ * 256), np.float32)
        for b in range(BPC):
            g = c * BPC + b
            idx = np.flatnonzero(~key_mask[g])
            ci = len(idx)
            quesT[32 * b:32 * b + 32] = ques[g].T
            keysT[32 * b:32 * b + 32, :ci] = keys[g][idx].T
            vc = np.zeros((KC, D), np.float32)
            vc[:ci] = vals[g][idx]
            ones = np.zeros((KC,), np.float32)
            ones[:ci] = 1.0
            for j in range(NJ):
                valsP[:, 256 * j + 64 * b:256 * j + 64 * b + 32] = \
                    vc[128 * j:128 * (j + 1)]
                valsP[:, 256 * j + 64 * b + 32] = ones[128 * j:128 * (j + 1)]
        import ml_dtypes
        bf = ml_dtypes.bfloat16
        in_maps.append({
            "quesT": quesT, "quesTb": quesT.astype(bf),
            "keysTb": keysT.astype(bf), "valsPb": valsP.astype(bf),
            "wq_st": wq_st, "wk_st": wk_st, "wv_st": wv_st,
            "ind_sig": indsig, "ind_sq": indsq, "ind_b": indb,
            "ind_21": ind21, "ind_s1": inds1, "ind_m4": indm4,
            "ind_g4": indg4,
        })
    return in_maps, KC


def unshard_output(results):
    out = np.empty((B, LQ, D), np.float32)
    for c in range(NCORES):
        o = results[c]["out"]
        for b in range(BPC):
            out[c * BPC + b] = o[32 * b:32 * b + 32, :].T
    return out


def kernel(**inputs) -> np.ndarray:
    in_maps, KC = prepare_inputs(**inputs)
    key = ("nc", KC)
    if key not in _cache:
        _cache[key] = build_module(KC)
    nc = _cache[key]
    res = bass_utils.run_bass_kernel_spmd(nc, in_maps,
                                          core_ids=list(range(NCORES)))
    return unshard_output(res.results)
